# revision 13
# baseline (speedup 1.0000x reference)
"""Trainium2 Bass kernel for nn_BiasedMultiHeadAttention (B=4, H=16, L=1024, E=1024).

Sharding: 64 (batch, head) pairs over 8 cores -> core c handles batch b=c//2,
heads h0=(c%2)*8 .. h0+8. Each core runs LayerNorm + its Q/K/V projection
slices + biased masked attention for its 8 heads + its slice of the output
projection (row-parallel). The two cores sharing a batch each return a partial
[L, E] out-projection; the host sums the pair and adds residual + bo.

Host-side folding (exact algebra, done in fp32):
  - gamma/beta folded into the projection weights/biases
  - 1/sqrt(D) folded into Wq/bq
  - gate*bias pre-exponentiated: device computes exp(Q K^T) * egb where
    egb = exp(gate*bias) * keymask * querymask (softmax shift/scale cancels in
    the normalization, and masking becomes an exact multiply-by-zero)
  - an epsilon row seeds the softmax denominator so fully-masked query columns
    normalize to exactly 0 instead of NaN.

Device layouts (per core): attention runs transposed, logitsT[k, q], so the
softmax denominator falls out of the attention*V matmul via an appended
ones-column on V, and the key mask rides for free inside egb.
"""
import numpy as np
import ml_dtypes
from contextlib import ExitStack

import concourse.bass as bass
import concourse.bacc as bacc
import concourse.tile as tile
from concourse import mybir
from concourse.bass_utils import run_bass_kernel_spmd

BF16 = mybir.dt.bfloat16
F32 = mybir.dt.float32
NBF16 = ml_dtypes.bfloat16
AF = mybir.ActivationFunctionType
ALU = mybir.AluOpType

P = 128
B, L, E, D, H = 4, 1024, 1024, 64, 16
HPC = 8            # heads per core
FL = HPC * D       # local feature width = 512
FC = FL // P       # 4 feature chunks
EC = E // P        # 8 embed chunks
LC = L // P        # 8 sequence chunks
NCORES = 8
LN_EPS = 1e-5

_NC = None


def _emit(nc, tc, ctx, xd, wq_d, wk_d, wv_d, wo_d, bq_d, bk_d, bv_d, eg_d, id_d,
          sel_d, out_d):
    sync = nc.sync
    x_t = xd.ap().rearrange("(t p) e -> t p e", p=P)
    out_t = out_d.ap().rearrange("(t p) e -> t p e", p=P)

    consts = ctx.enter_context(tc.tile_pool(name="consts", bufs=1))
    dramp = ctx.enter_context(tc.tile_pool(name="scratch", bufs=1, space="DRAM"))

    # x tiles first: the LayerNorm -> transpose -> projection critical path
    # starts with them, so they must win the early DMA bandwidth
    xts = []
    xpool = ctx.enter_context(tc.tile_pool(name="xin", bufs=1))
    for t in range(LC):
        xt = xpool.tile([P, E], F32, tag=f"x{t}")
        sync.dma_start(xt[:], x_t[t])
        xts.append(xt)
    ident = consts.tile([P, P], BF16)
    sync.dma_start(ident[:], id_d.ap())
    eps_ln = consts.tile([P, 1], F32)
    nc.vector.memset(eps_ln[:], LN_EPS)
    onescol = consts.tile([1, P], BF16)
    nc.vector.memset(onescol[:], 1.0)
    # selectors for the denominator broadcast: cols 0:128 select partitions
    # 0:64 (head A), cols 128:256 select partitions 64:128 (head B); each is
    # used as a rank-1 matmul lhsT to replicate a row across its partitions
    sel2 = consts.tile([1, 2 * P], BF16)
    sync.dma_start(sel2[:], sel_d.ap())
    bvr = consts.tile([1, FL], BF16)
    sync.dma_start(bvr[:], bv_d.ap())
    bqc = consts.tile([P, FC], F32)
    sync.dma_start(bqc[:], bq_d.ap())
    bkc = consts.tile([P, FC], F32)
    sync.dma_start(bkc[:], bk_d.ap())
    wo_sb = consts.tile([P, FC, E], BF16)

    xhatT = consts.tile([P, EC, L], BF16)   # xhat transposed: [e, l]
    qT = consts.tile([P, FC, L], BF16)      # Q^T: [f, l] (scale folded in)
    kT = consts.tile([P, FC, L], BF16)      # K^T: [f, l]
    vaug = consts.tile([P, LC, HPC, 65], BF16)  # V | ones column, per l-chunk/head
    otun = consts.tile([P, FC, L], BF16)    # unnormalized attention output^T
    otall = consts.tile([P, FC, L], BF16)   # normalized attention output^T
    nc.vector.memset(vaug[:, :, :, 64:65], 1.0)


    # ---- Phases A+B interleaved: LayerNorm + PE transposes + projections ----
    # Emission order matters: the PE stream is in-order, so projections over
    # the first half of the sequence are emitted right after LN tiles 0-3,
    # keeping PE dense (and HAM warm) while LN tiles 4-7 still stream.
    with tc.tile_pool(name="stats", bufs=6) as statp, \
         tc.tile_pool(name="xh", bufs=3) as xhp, \
         tc.tile_pool(name="w", bufs=1) as wpool, \
         tc.tile_pool(name="tp", bufs=2, space="PSUM") as tpp, \
         tc.tile_pool(name="pjqk", bufs=4, space="PSUM") as pjqk, \
         tc.tile_pool(name="pjv", bufs=2, space="PSUM") as pjv:
        wq_sb = wpool.tile([P, EC, FL], BF16)
        sync.dma_start(wq_sb[:], wq_d.ap())
        wk_sb = wpool.tile([P, EC, FL], BF16)
        sync.dma_start(wk_sb[:], wk_d.ap())
        wv_sb = wpool.tile([P, EC, FL], BF16)
        sync.dma_start(wv_sb[:], wv_d.ap())

        def ln_tile(t):
            xt = xts[t]
            st = statp.tile([P, 2, 6], F32)
            nc.vector.bn_stats(st[:, 0, :], xt[:, 0:512])
            nc.vector.bn_stats(st[:, 1, :], xt[:, 512:1024])
            mv = statp.tile([P, 2], F32)
            nc.vector.bn_aggr(mv[:], st[:])
            srt = statp.tile([P, 1], F32)
            nc.scalar.activation(srt[:], mv[:, 1:2], AF.Sqrt, bias=eps_ln[:],
                                 scale=1.0)
            rstd = statp.tile([P, 1], F32)
            nc.vector.reciprocal(rstd[:], srt[:])
            xh = xhp.tile([P, E], BF16)
            nc.vector.tensor_scalar(xh[:], xt[:], mv[:, 0:1], rstd[:],
                                    op0=ALU.subtract, op1=ALU.mult)
            # transpose each [128,128] block on the (otherwise idle) PE
            for et in range(EC):
                tp = tpp.tile([P, P], BF16)
                nc.tensor.transpose(tp[:], xh[:, bass.ts(et, P)], ident[:])
                if et % 2 == 0:
                    nc.scalar.copy(xhatT[:, et, bass.ts(t, P)], tp[:])
                else:
                    nc.vector.tensor_copy(xhatT[:, et, bass.ts(t, P)], tp[:])

        def proj_half(half):
            for fc in range(FC):
                for w_sb, dest, bcol in ((wq_sb, qT, bqc), (wk_sb, kT, bkc)):
                    ps = pjqk.tile([P, 512], F32)
                    for ec in range(EC):
                        nc.tensor.matmul(
                            ps[:],
                            lhsT=w_sb[:, ec, fc * P:(fc + 1) * P],
                            rhs=xhatT[:, ec, half * 512:(half + 1) * 512],
                            start=(ec == 0), stop=(ec == EC - 1))
                    nc.scalar.activation(dest[:, fc, half * 512:(half + 1) * 512],
                                         ps[:], AF.Identity,
                                         bias=bcol[:, fc:fc + 1], scale=1.0)
            for lc in range(half * 4, half * 4 + 4):
                ps = pjv.tile([P, FL], F32)
                nc.tensor.matmul(ps[:], lhsT=onescol[:], rhs=bvr[:],
                                 start=True, stop=False)
                for ec in range(EC):
                    nc.tensor.matmul(ps[:], lhsT=xhatT[:, ec, bass.ts(lc, P)],
                                     rhs=wv_sb[:, ec, :],
                                     start=False, stop=(ec == EC - 1))
                nc.vector.tensor_copy(vaug[:, lc, :, 0:64],
                                      ps[:].rearrange("p (h d) -> p h d", h=HPC))

        for t in range(4):
            ln_tile(t)
        proj_half(0)
        for t in range(4, LC):
            ln_tile(t)
        proj_half(1)

    # out-projection weights aren't needed until the very end; load them
    # once the front-critical DMAs have been issued
    sync.dma_start(wo_sb[:], wo_d.ap())

    # ---- Phase C: attention, one head at a time, transposed layout ----
    with tc.tile_pool(name="egb", bufs=4) as egbp, \
         tc.tile_pool(name="attn", bufs=3) as atp, \
         tc.tile_pool(name="rows", bufs=4) as rowp, \
         tc.tile_pool(name="lg", bufs=1, space="PSUM") as lg, \
         tc.tile_pool(name="otp", bufs=1, space="PSUM") as otp:
        # Heads are processed in pairs (hA at partitions 0:64, hB at 64:128 of
        # the shared fc chunk). Both heads' logits land in one [128, 2048]
        # 4-bank PSUM tile so exp and the egb multiply run as single wide
        # instructions (amortizing per-instruction overhead), and the AV
        # matmuls are software-pipelined one kc tick behind QK so the PE
        # stream never waits on the exp->mul chain and HAM keeps 2.4 GHz.
        def qk_pair(fc, kc):
            lgt = lg.tile([P, 2 * L], F32, tag="lgAB")
            for po in (0, 64):
                for half in range(2):
                    o = po * 16 + half * 512
                    nc.tensor.matmul(
                        lgt[:, o:o + 512],
                        lhsT=kT[po:po + 64, fc, bass.ts(kc, P)],
                        rhs=qT[po:po + 64, fc, half * 512:(half + 1) * 512],
                        start=True, stop=True)
            return lgt

        def av_pair(pend):
            otA, otB, at, kc, hA, hB = pend
            for ot_ps, h, o0 in ((otA, hA, 0), (otB, hB, L)):
                for half in range(2):
                    nc.tensor.matmul(
                        ot_ps[:, half * 512:(half + 1) * 512],
                        lhsT=vaug[:, kc, h, :],
                        rhs=at[:, o0 + half * 512:o0 + half * 512 + 512],
                        start=(kc == 0), stop=(kc == LC - 1))

        for hp in range(HPC // 2):
            hA, hB, fc = 2 * hp, 2 * hp + 1, hp
            # QK for kc=0 first: it has no dependency on the previous pair's
            # OT drain, so the PE stream rolls across the pair boundary.
            lgt = qk_pair(fc, 0)
            otA = otp.tile([65, L], F32, tag="otA")
            otB = otp.tile([65, L], F32, tag="otB")
            pend = None
            for kc in range(LC):
                if kc > 0:
                    lgt = qk_pair(fc, kc)
                egbt = egbp.tile([P, 2 * L], BF16, tag="egbAB")
                sync.dma_start(egbt[:], eg_d.ap()[hp, kc])
                el = atp.tile([P, 2 * L], BF16, tag="elAB")
                nc.scalar.activation(el[:], lgt[:], AF.Exp)
                at = atp.tile([P, 2 * L], BF16, tag="atAB")
                nc.vector.tensor_mul(at[:], el[:], egbt[:])
                if pend is not None:
                    av_pair(pend)
                pend = (otA, otB, at, kc, hA, hB)
            av_pair(pend)
            # per-head normalization, fully on-chip: copy the denominator
            # row from psum partition 64 down to a [2, L] staging tile,
            # approx-reciprocal it (+eps so fully-masked query columns give 0,
            # not NaN), then broadcast both heads' rows across partitions with
            # one rank-2 matmul (sel2) and multiply.
            rrt = []
            for h, ot_ps in ((hA, otA), (hB, otB)):
                po = (h % 2) * 64
                nc.vector.tensor_copy(otun[po:po + 64, fc, :], ot_ps[0:64, :])
                s0 = rowp.tile([1, L], F32, tag="s0")
                if h % 2 == 0:
                    nc.scalar.copy(s0[:], ot_ps[64:65, :])
                else:
                    nc.vector.tensor_copy(s0[:], ot_ps[64:65, :])
                r0 = rowp.tile([1, L], F32, tag="r0")
                nc.vector.tensor_scalar_add(r0[:], s0[:], 1e-20)
                rr = rowp.tile([1, L], F32, tag="rr")
                nc.vector.reciprocal_approx_fast(rr[:], r0[:])
                rb = rowp.tile([1, L], BF16, tag=f"rr{h % 2}")
                nc.vector.tensor_copy(rb[:], rr[:])
                rrt.append(rb)
            qsb_ps = otp.tile([P, L], F32, tag="otA")
            for half in range(2):
                nc.tensor.matmul(qsb_ps[:, half * 512:(half + 1) * 512],
                                 lhsT=sel2[0:1, 0:P],
                                 rhs=rrt[0][0:1, half * 512:(half + 1) * 512],
                                 start=True, stop=False)
                nc.tensor.matmul(qsb_ps[:, half * 512:(half + 1) * 512],
                                 lhsT=sel2[0:1, P:2 * P],
                                 rhs=rrt[1][0:1, half * 512:(half + 1) * 512],
                                 start=False, stop=True)
            nc.vector.tensor_mul(otall[:, fc, :], otun[:, fc, :], qsb_ps[:])

    # ---- Phase D: output projection (partial, host adds residual+bo and pairs) ----
    with tc.tile_pool(name="op", bufs=2, space="PSUM") as op, \
         tc.tile_pool(name="outs", bufs=3) as outp:
        for lc in range(LC):
            ps = op.tile([P, E], F32)
            for half in range(2):
                for fc in range(FC):
                    nc.tensor.matmul(
                        ps[:, half * 512:(half + 1) * 512],
                        lhsT=otall[:, fc, bass.ts(lc, P)],
                        rhs=wo_sb[:, fc, half * 512:(half + 1) * 512],
                        start=(fc == 0), stop=(fc == FC - 1))
            ot = outp.tile([P, E], F32)
            nc.scalar.copy(ot[:, 0:512], ps[:, 0:512])
            nc.vector.tensor_copy(ot[:, 512:1024], ps[:, 512:1024])
            sync.dma_start(out_t[lc], ot[:])


def build_nc():
    nc = bacc.Bacc("TRN2", target_bir_lowering=False, debug=False)
    xd = nc.dram_tensor("x", [L, E], F32, kind="ExternalInput")
    wq_d = nc.dram_tensor("wqT", [P, EC, FL], BF16, kind="ExternalInput")
    wk_d = nc.dram_tensor("wkT", [P, EC, FL], BF16, kind="ExternalInput")
    wv_d = nc.dram_tensor("wvT", [P, EC, FL], BF16, kind="ExternalInput")
    wo_d = nc.dram_tensor("woT", [P, FC, E], BF16, kind="ExternalInput")
    bq_d = nc.dram_tensor("bqc", [P, FC], F32, kind="ExternalInput")
    bk_d = nc.dram_tensor("bkc", [P, FC], F32, kind="ExternalInput")
    bv_d = nc.dram_tensor("bvr", [1, FL], BF16, kind="ExternalInput")
    eg_d = nc.dram_tensor("egb", [HPC // 2, LC, P, 2 * L], BF16, kind="ExternalInput")
    id_d = nc.dram_tensor("ident", [P, P], BF16, kind="ExternalInput")
    sel_d = nc.dram_tensor("sel2", [1, 2 * P], BF16, kind="ExternalInput")
    out_d = nc.dram_tensor("partial", [L, E], F32, kind="ExternalOutput")
    with tile.TileContext(nc) as tc, ExitStack() as ctx:
        _emit(nc, tc, ctx, xd, wq_d, wk_d, wv_d, wo_d, bq_d, bk_d, bv_d, eg_d,
              id_d, sel_d, out_d)
    nc.compile()
    return nc


def _wdev(w):
    # [FL, E] slice of an LN-folded weight -> lhsT layout [P, EC, FL]
    return np.ascontiguousarray(
        w.T.reshape(EC, P, FL).transpose(1, 0, 2)).astype(NBF16)


def prepare_in_maps(x, bias, mask, Wq, bq, Wk, bk, Wv, bv, Wo, bo, gamma, beta, gate):
    x = np.asarray(x, np.float32)
    gamma = np.asarray(gamma, np.float32)
    beta = np.asarray(beta, np.float32)
    gate = np.asarray(gate, np.float32)
    Wq = np.asarray(Wq, np.float32)
    Wk = np.asarray(Wk, np.float32)
    Wv = np.asarray(Wv, np.float32)
    Wo = np.asarray(Wo, np.float32)
    bq = np.asarray(bq, np.float32)
    bk = np.asarray(bk, np.float32)
    bv = np.asarray(bv, np.float32)
    scale = 1.0 / np.sqrt(np.float32(D))

    Wqe = (Wq * gamma[None, :]) * scale
    Wke = Wk * gamma[None, :]
    Wve = Wv * gamma[None, :]
    bqe = (bq + Wq @ beta) * scale
    bke = bk + Wk @ beta
    bve = bv + Wv @ beta
    mf = np.asarray(mask, np.float32)

    in_maps = []
    for c in range(NCORES):
        b, h0 = c // 2, (c % 2) * HPC
        sl = slice(h0 * D, h0 * D + FL)
        g = gate[h0:h0 + HPC]
        bb = np.asarray(bias[b, h0:h0 + HPC], np.float32)      # [HPC, q, k]
        egb = np.exp(g[:, None, None] * bb)
        egb *= mf[b][None, None, :]                            # key mask
        egb *= mf[b][None, :, None]                            # query mask
        egbT = (np.ascontiguousarray(egb.transpose(0, 2, 1))
                .reshape(HPC // 2, 2, LC, P, L).transpose(0, 2, 3, 1, 4)
                .reshape(HPC // 2, LC, P, 2 * L))
        egbT = np.ascontiguousarray(egbT)
        in_maps.append({
            "x": np.ascontiguousarray(x[b]),
            "wqT": _wdev(Wqe[sl]),
            "wkT": _wdev(Wke[sl]),
            "wvT": _wdev(Wve[sl]),
            "woT": np.ascontiguousarray(
                Wo[:, sl].T.reshape(FC, P, E).transpose(1, 0, 2)).astype(NBF16),
            "bqc": np.ascontiguousarray(bqe[sl].reshape(FC, P).T),
            "bkc": np.ascontiguousarray(bke[sl].reshape(FC, P).T),
            "bvr": bve[sl].reshape(1, FL).astype(NBF16),
            "egb": egbT.astype(NBF16),
            "ident": np.eye(P, dtype=NBF16),
            "sel2": np.kron(np.eye(2), np.ones((1, 64))).reshape(1, 2 * P).astype(NBF16),
        })
    return in_maps


def finish(x, bo, partials):
    x = np.asarray(x, np.float32)
    bo = np.asarray(bo, np.float32)
    out = np.empty((B, L, E), np.float32)
    for b in range(B):
        out[b] = x[b] + partials[2 * b] + partials[2 * b + 1] + bo[None, :]
    return out


def run_spmd(in_maps, trace=False, trace_cores=None, **kw):
    global _NC
    if _NC is None:
        _NC = build_nc()
    return run_bass_kernel_spmd(_NC, in_maps, core_ids=list(range(NCORES)),
                                trace=trace, trace_cores=trace_cores, **kw)


def kernel(**inputs):
    in_maps = prepare_in_maps(**inputs)
    res = run_spmd(in_maps)
    partials = [r["partial"] for r in res.results]
    return finish(inputs["x"], inputs["bo"], partials)

